# revision 2
# baseline (speedup 1.0000x reference)
"""2-layer GCN (GCNConv x2 + log_softmax) on 8 Trainium2 NeuronCores — v2.

out = log_softmax( A_hat @ relu(A_hat @ (x@W1) + b1) @ W2 + b2 )
with A_hat = D^-1/2 (A + I) D^-1/2.

v2 replaces the per-edge HBM dma_gather (GPSIMD Q7 descriptor generation was
77% of runtime) with SBUF-resident ap_gather in a FEATURE-MAJOR layout:

  * The node-feature table lives transposed in SBUF as [128, padloc] f32:
    partition 16*g+f holds feature f of core g's nodes (free dim = node).
    One AllGather of the local [16, padloc] dump materializes exactly this
    layout.
  * ap_gather runs on all 8 Q7 cores in parallel; group g (partitions
    16g..16g+15) gathers columns (= node vectors) with its own index list.
    Group g owns the edges whose SRC belongs to core g; each core owns the
    edges whose DST it holds.
  * Scatter-free segment-sum: per (core, group), dst nodes are ranked by
    in-degree; occurrence round r gathers the r-th message for ranks
    [0, n_r) which lands as a contiguous COLUMN block, so accumulation is a
    dense DVE add onto an accumulator prefix. Round sizes are maxed across
    all (core, group) instances (shortfall gathers a zeroed dead column), so
    one SPMD program serves all cores.
  * Rank->node merge: one ap_gather per group over the accumulator, then the
    8 groups are summed with a single [128]->[16] TensorE matmul (selection
    matrix B); + the self-loop row gives tot = A_hat-propagated features.
  * Both propagations run at width 16 (A_hat(zW2) == (A_hat z)W2); layer 2
    reuses identical gather indices. The final W2 matmul emits node-major
    [128, 40] tiles directly (lhsT = feature-major slice), then log_softmax.
"""

import os
import sys
from contextlib import ExitStack

import numpy as np

if "/opt/trn_rl_repo" not in sys.path:
    sys.path.insert(0, "/opt/trn_rl_repo")

# ---------------------------------------------------------------- constants
N_NODES = 100000
NCORES = 8
NGRP = 8
F_IN = 512
HID = 16
NCLS = 40
P = 128
S = 5632          # gather idxs per group per ap_gather call
SIDX = S // 16
MCOL = 448        # merge matmul column tile (psum: 448*4B = 1792B)

LAST_EXEC_NS = None


def _dims():
    nloc = N_NODES // NCORES
    tslot = -(-nloc // P)
    return nloc, tslot, tslot * P


def _wrap16(flat):
    """idx vector (len % 16 == 0) -> [16, n/16] per-group ap_gather layout."""
    n = flat.size
    assert n % 16 == 0
    return flat.reshape(n // 16, 16).T.astype(np.int16)


# ================================================================ host plan
def _plan(edge_index):
    nloc, tslot, padloc = _dims()
    dead = padloc - 1                     # zeroed pad column
    assert padloc - 1 <= np.iinfo(np.int16).max
    assert padloc > nloc

    src = np.asarray(edge_index[0]).astype(np.int64)
    dst = np.asarray(edge_index[1]).astype(np.int64)
    owner = dst // nloc
    gsrc = src // nloc
    scol = src % nloc
    ldst = dst % nloc

    entries = [[None] * NGRP for _ in range(NCORES)]
    nrounds = np.zeros((NCORES, NGRP), np.int64)
    midx = np.empty((NCORES, P, padloc // 16), np.int16)
    disT = np.zeros((NCORES, HID, padloc), np.float32)

    for c in range(NCORES):
        m = owner == c
        g_c, s_c, l_c = gsrc[m], scol[m], ldst[m]
        deg_tot = np.bincount(l_c, minlength=nloc).astype(np.float64) + 1.0
        disT[c, :, :nloc] = (1.0 / np.sqrt(deg_tot)).astype(np.float32)[None, :]

        for g in range(NGRP):
            mg = g_c == g
            s_q, l_q = s_c[mg], l_c[mg]
            deg_q = np.bincount(l_q, minlength=nloc)
            order_q = np.argsort(-deg_q, kind="stable")
            rank_of = np.empty(nloc, np.int64)
            rank_of[order_q] = np.arange(nloc)
            r_e = rank_of[l_q]
            o1 = np.argsort(r_e, kind="stable")
            rs, ss = r_e[o1], s_q[o1]
            deg_rank = deg_q[order_q]
            starts = np.zeros(nloc, np.int64)
            starts[1:] = np.cumsum(deg_rank)[:-1]
            occ = np.arange(rs.size, dtype=np.int64) - starts[rs]
            o2 = np.argsort(occ * nloc + rs, kind="stable")
            rows_sorted = ss[o2].astype(np.int16)
            n_r = (np.bincount(occ).astype(np.int64) if occ.size
                   else np.zeros(0, np.int64))
            offs = np.zeros(n_r.size + 1, np.int64)
            offs[1:] = np.cumsum(n_r)
            entries[c][g] = (rows_sorted, n_r, offs)
            nrounds[c, g] = n_r.size

            mi = np.full(padloc, dead, np.int64)
            mi[:nloc] = rank_of
            midx[c, 16 * g:16 * (g + 1)] = _wrap16(mi)

    # unified round sizes (max over cores and groups)
    R = int(nrounds.max())
    n_r_max = np.zeros(R, np.int64)
    for c in range(NCORES):
        for g in range(NGRP):
            n_r = entries[c][g][1]
            n_r_max[: n_r.size] = np.maximum(n_r_max[: n_r.size], n_r)
    t_starts = np.zeros(R + 1, np.int64)
    t_starts[1:] = np.cumsum(n_r_max)
    T = int(t_starts[R])
    NB = -(-T // S)
    # all batches are S idxs except a trimmed last one (16-aligned)
    batch_sizes = [S] * (NB - 1) + [-(-(T - (NB - 1) * S) // 16) * 16]
    bounds = np.zeros(NB + 1, np.int64)
    bounds[1:] = np.cumsum(batch_sizes)
    Tpad = int(bounds[NB])

    # add schedule: per batch, (acc_offset, staging_offset, len) segments
    addsched = [[] for _ in range(NB)]
    for r in range(R):
        t0, n = int(t_starts[r]), int(n_r_max[r])
        pos = t0
        while pos < t0 + n:
            b = int(np.searchsorted(bounds, pos, side="right")) - 1
            ln = min(int(bounds[b + 1]), t0 + n) - pos
            addsched[b].append((pos - t0, pos - int(bounds[b]), int(ln)))
            pos += ln

    gidx = np.empty((NCORES, NB, P, SIDX), np.int16)
    for c in range(NCORES):
        for g in range(NGRP):
            rows_sorted, n_r, offs = entries[c][g]
            stream = np.full(Tpad, dead, np.int64)
            for r in range(n_r.size):
                v = rows_sorted[offs[r]: offs[r + 1]]
                stream[t_starts[r]: t_starts[r] + v.size] = v
            for b in range(NB):
                blk = np.full(S, dead, np.int64)
                blk[: batch_sizes[b]] = stream[bounds[b]: bounds[b + 1]]
                gidx[c, b, 16 * g:16 * (g + 1)] = _wrap16(blk)

    B = np.zeros((P, HID), np.float32)
    B[np.arange(P), np.arange(P) % HID] = 1.0

    meta = dict(NB=NB, addsched=addsched, batch_sizes=batch_sizes,
                tslot=tslot, nloc=nloc, padloc=padloc,
                # legacy keys for test.py prints
                ng=NB, batches=[(0, sched) for sched in addsched])
    host = dict(gidx=gidx, midx=midx, disT=disT, B=B)
    return meta, host


# ============================================================ device program
def _emit(tc, io, meta, reps=1):
    from concourse import mybir

    nc = tc.nc
    f32 = mybir.dt.float32
    bf16 = mybir.dt.bfloat16
    i16 = mybir.dt.int16
    nloc, tslot, padloc = meta["nloc"], meta["tslot"], meta["padloc"]
    NB, addsched = meta["NB"], meta["addsched"]
    kch = F_IN // P
    AF = mybir.ActivationFunctionType
    AL = mybir.AluOpType

    with ExitStack() as ctx:
        sb = ctx.enter_context(tc.tile_pool(name="sb", bufs=1))
        xb = ctx.enter_context(tc.tile_pool(name="xb", bufs=2))
        stg = ctx.enter_context(tc.tile_pool(name="stg", bufs=2))
        ib = ctx.enter_context(tc.tile_pool(name="ib", bufs=2))
        ps = ctx.enter_context(tc.tile_pool(name="ps", bufs=4, space="PSUM"))
        ps1 = ctx.enter_context(tc.tile_pool(name="ps1", bufs=1, space="PSUM"))
        dram = ctx.enter_context(tc.tile_pool(name="dram", bufs=1, space="DRAM"))

        # ---- persistent small tiles
        w1t = sb.tile([P, kch * HID], bf16, tag="w1")
        nc.sync.dma_start(
            w1t[:].rearrange("p (k h) -> p k h", h=HID),
            io["W1"].rearrange("(k p) h -> p k h", p=P),
        )
        w2t = sb.tile([HID, NCLS], f32, tag="w2")
        nc.sync.dma_start(w2t[:], io["W2"])
        b1T = sb.tile([HID, 1], f32, tag="b1T")
        nc.sync.dma_start(b1T[:], io["b1"].rearrange("o h -> h o"))

        ones1 = sb.tile([1, P], f32, tag="ones1")
        nc.vector.memset(ones1[:], 1.0)
        b2s = sb.tile([1, NCLS], f32, tag="b2s")
        nc.sync.dma_start(b2s[:], io["b2"])
        b2p = ps1.tile([P, NCLS], f32, tag="biasp")
        nc.tensor.matmul(b2p[:], lhsT=ones1[:], rhs=b2s[:], start=True, stop=True)
        b2bc = sb.tile([P, NCLS], f32, tag="b2bc")
        nc.vector.tensor_copy(b2bc[:], b2p[:])

        Bt = sb.tile([P, HID], f32, tag="Bt")
        nc.sync.dma_start(Bt[:], io["B"])

        # Scratch bands. Verifier rules: engine AP base partition must be
        # 0/32/64/96, and two SBUF inputs of a DVE op must share the base —
        # so every two-SBUF-input op here pairs equal bases, everything else
        # goes through PSUM (mixed PSUM+SBUF inputs are exempt).
        #   rows 0:16  = hh -> z -> p  (sequential reuse, base 0)
        #   rows 64:80 = disT
        SM = sb.tile([P, padloc], f32, tag="SM")
        nc.sync.dma_start(SM[64:80, :], io["disT"])
        TB = sb.tile([P, padloc], f32, tag="TB")
        ACC = sb.tile([P, padloc], f32, tag="ACC")

        mx = sb.tile([P, tslot], f32, tag="mx")
        sm_ = sb.tile([P, tslot], f32, tag="sm")
        ls = sb.tile([P, tslot], f32, tag="ls")

        hhd = dram.tile([HID, padloc], f32, tag="hhd")
        ztd = dram.tile([HID, padloc], f32, tag="ztd")
        table1 = dram.tile([P, padloc], f32, tag="table1", addr_space="Shared")
        table2 = dram.tile([P, padloc], f32, tag="table2", addr_space="Shared")

        def a3(t):
            return t[:].rearrange("p (n d) -> p n d", d=1)

        def allgather(local_dram, table_dram):
            nc.gpsimd.collective_compute(
                "AllGather", AL.bypass,
                replica_groups=[list(range(NCORES))],
                ins=[local_dram[:].opt()], outs=[table_dram[:].opt()],
            )

        for _rep in range(reps):
            # ---- phase A: hh^T = disT * (x @ W1)^T  (feature-major)
            # 4 node-tiles share one PSUM bank (independent col ranges), one
            # dis-scale DVE op per 512 cols.
            CB = 4
            nc.vector.memset(SM[0:16, :], 0.0)
            for t0 in range(0, tslot, CB):
                tn = min(CB, tslot - t0)
                hp = ps.tile([HID, CB * P], f32, tag="u")
                for ti in range(tn):
                    t = t0 + ti
                    w = min(P, nloc - t * P)
                    xt = xb.tile([P, kch * P], bf16, tag="xt")
                    nc.sync.dma_start(
                        xt[:, : kch * w].rearrange("p (k n) -> p k n", k=kch),
                        io["xT"][:, t * P: t * P + w].rearrange(
                            "(k p) n -> p k n", p=P),
                    )
                    for k in range(kch):
                        nc.tensor.matmul(
                            hp[:, ti * P: ti * P + w],
                            lhsT=w1t[:, k * HID: (k + 1) * HID],
                            rhs=xt[:, k * w: (k + 1) * w],
                            start=(k == 0),
                            stop=(k == kch - 1),
                        )
                cw = min(CB * P, nloc - t0 * P)
                nc.vector.tensor_tensor(
                    SM[0:16, t0 * P: t0 * P + cw], hp[:, :cw],
                    SM[64:80, t0 * P: t0 * P + cw], op=AL.mult)

            def propagate(layer):
                """Gather + segment-sum + merge; leaves the layer's output
                (z for layer 1, p for layer 2) in SM[0:16], consuming the
                self/table source previously there (hh or z)."""
                nc.vector.memset(ACC[:], 0.0)
                for b in range(NB):
                    bs = meta["batch_sizes"][b]
                    it = ib.tile([P, SIDX], i16, tag="it")
                    nc.sync.dma_start(it[:, : bs // 16],
                                      io["gidx"][b][:, : bs // 16])
                    st = stg.tile([P, S], f32, tag="st")
                    nc.gpsimd.ap_gather(
                        out_ap=a3(st[:, :bs]), in_ap=a3(TB),
                        idxs_ap=it[:, : bs // 16],
                        channels=P, num_elems=padloc, d=1, num_idxs=bs,
                    )
                    for (a0, o, ln) in addsched[b]:
                        nc.vector.tensor_add(
                            ACC[:, a0:a0 + ln], ACC[:, a0:a0 + ln],
                            st[:, o:o + ln])
                # merge rank->node (ap_gather staged into TB, which is dead
                # by now), sum the 8 groups (TensorE), add self, and apply
                # the layer's dis-scaling chain per MCOL chunk. Two halves so
                # the first half's chunk chain overlaps the second gather.
                # PSUM-mixed inputs dodge the equal-base-partition rule.
                mit = ib.tile([P, padloc // 16], i16, tag="it")
                nc.sync.dma_start(mit[:], io["midx"])
                half = (padloc // 2 + 15) // 16 * 16 if padloc > 2 * MCOL else padloc
                for (m0, m1) in ([(0, half), (half, padloc)]
                                 if half < padloc else [(0, padloc)]):
                    nc.gpsimd.ap_gather(
                        out_ap=a3(TB[:, m0:m1]), in_ap=a3(ACC),
                        idxs_ap=mit[:, m0 // 16: m1 // 16],
                        channels=P, num_elems=padloc, d=1, num_idxs=m1 - m0,
                    )
                    for q in range(-(-(m1 - m0) // MCOL)):
                        c0 = m0 + q * MCOL
                        cw = min(MCOL, m1 - c0)
                        sl = slice(c0, c0 + cw)
                        pm = ps.tile([HID, MCOL], f32, tag="u")
                        nc.tensor.matmul(pm[:, :cw], lhsT=Bt[:], rhs=TB[:, sl],
                                         start=True, stop=True)
                        pm2 = ps.tile([HID, MCOL], f32, tag="u")
                        nc.vector.tensor_add(pm2[:, :cw], pm[:, :cw],
                                             SM[0:16, sl])
                        # overwrite self with dis * (merged + self)
                        nc.vector.tensor_tensor(SM[0:16, sl], pm2[:, :cw],
                                                SM[64:80, sl], op=AL.mult)
                        if layer == 1:
                            # z = dis * relu(<prev> + b1)
                            nc.scalar.activation(SM[0:16, sl], SM[0:16, sl],
                                                 AF.Relu, bias=b1T[:])
                            cpd = ps.tile([HID, MCOL], f32, tag="u")
                            nc.vector.tensor_copy(cpd[:, :cw], SM[64:80, sl])
                            nc.vector.tensor_tensor(SM[0:16, sl],
                                                    SM[0:16, sl],
                                                    cpd[:, :cw], op=AL.mult)

            # ---- layer 1  (SM[0:16]: hh -> z)
            nc.sync.dma_start(hhd[:], SM[0:16, :])
            allgather(hhd, table1)
            nc.sync.dma_start(TB[:], table1[:])
            propagate(1)

            # ---- layer 2  (SM[0:16]: z -> p)
            nc.sync.dma_start(ztd[:], SM[0:16, :])
            allgather(ztd, table2)
            nc.sync.dma_start(TB[:], table2[:])
            propagate(2)
            lg = ACC[:, : tslot * NCLS]
            ex = ACC[:, tslot * NCLS: 2 * tslot * NCLS]
            for t in range(tslot):
                w = min(P, nloc - t * P)
                lp = ps.tile([P, NCLS], f32, tag="u")
                nc.tensor.matmul(lp[:w, :], lhsT=SM[0:16, t * P: t * P + w],
                                 rhs=w2t[:], start=True, stop=True)
                nc.vector.tensor_add(lg[:w, t * NCLS:(t + 1) * NCLS],
                                     lp[:w, :], b2bc[:w, :])

            lg3 = lg.rearrange("p (t c) -> p t c", c=NCLS)
            nc.vector.reduce_max(out=mx[:], in_=lg3, axis=mybir.AxisListType.X)
            nc.vector.tensor_tensor(
                lg3, lg3, mx[:].unsqueeze(2).to_broadcast([P, tslot, NCLS]),
                op=AL.subtract)
            nc.scalar.activation(ex, lg, AF.Exp)
            nc.vector.reduce_sum(
                out=sm_[:], in_=ex.rearrange("p (t c) -> p t c", c=NCLS),
                axis=mybir.AxisListType.X)
            nc.scalar.activation(ls[:], sm_[:], AF.Ln)
            nc.vector.tensor_tensor(
                lg3, lg3, ls[:].unsqueeze(2).to_broadcast([P, tslot, NCLS]),
                op=AL.subtract)
            nc.sync.dma_start(io["out_raw"], lg)


def build_nc(meta, reps=1):
    import concourse.bacc as bacc
    import concourse.tile as tile
    from concourse import mybir

    nloc, tslot, padloc = _dims()
    f32, i16 = mybir.dt.float32, mybir.dt.int16
    bf16 = mybir.dt.bfloat16
    NB = meta["NB"]

    nc = bacc.Bacc("TRN2", target_bir_lowering=False, debug=False,
                   num_devices=NCORES)
    io = {
        "xT": nc.dram_tensor("xT", [F_IN, nloc], bf16, kind="ExternalInput").ap(),
        "W1": nc.dram_tensor("W1", [F_IN, HID], bf16, kind="ExternalInput").ap(),
        "b1": nc.dram_tensor("b1", [1, HID], f32, kind="ExternalInput").ap(),
        "W2": nc.dram_tensor("W2", [HID, NCLS], f32, kind="ExternalInput").ap(),
        "b2": nc.dram_tensor("b2", [1, NCLS], f32, kind="ExternalInput").ap(),
        "disT": nc.dram_tensor("disT", [HID, padloc], f32,
                               kind="ExternalInput").ap(),
        "gidx": nc.dram_tensor("gidx", [NB, P, SIDX], i16,
                               kind="ExternalInput").ap(),
        "midx": nc.dram_tensor("midx", [P, padloc // 16], i16,
                               kind="ExternalInput").ap(),
        "B": nc.dram_tensor("B", [P, HID], f32, kind="ExternalInput").ap(),
        "out_raw": nc.dram_tensor("out_raw", [P, tslot * NCLS], f32,
                                  kind="ExternalOutput").ap(),
    }
    with tile.TileContext(nc) as tc:
        _emit(tc, io, meta, reps=reps)
    nc.compile()
    return nc


def make_in_maps(inputs, host):
    import ml_dtypes

    nloc, _, _ = _dims()
    x = np.asarray(inputs["x"], np.float32)
    W1 = np.ascontiguousarray(
        np.asarray(inputs["W1"], np.float32).astype(ml_dtypes.bfloat16))
    b1 = np.asarray(inputs["b1"], np.float32).reshape(1, HID)
    W2 = np.ascontiguousarray(np.asarray(inputs["W2"], np.float32))
    b2 = np.asarray(inputs["b2"], np.float32).reshape(1, NCLS)
    in_maps = []
    for c in range(NCORES):
        in_maps.append({
            "xT": np.ascontiguousarray(
                x[c * nloc: (c + 1) * nloc].T.astype(ml_dtypes.bfloat16)),
            "W1": W1, "b1": b1, "W2": W2, "b2": b2,
            "disT": host["disT"][c],
            "gidx": host["gidx"][c],
            "midx": host["midx"][c],
            "B": host["B"],
        })
    return in_maps


def unshard(results, host):
    nloc, tslot, padloc = _dims()
    out = np.empty((N_NODES, NCLS), np.float32)
    for c in range(NCORES):
        raw = results[c]["out_raw"].reshape(P, tslot, NCLS)
        out[c * nloc: (c + 1) * nloc] = (
            raw.transpose(1, 0, 2).reshape(padloc, NCLS)[:nloc])
    return out


def run_hw_timed(nc, in_maps, iters=6):
    """Device-resident wall-clock timing over repeated dispatches."""
    import time

    import jax
    from concourse import bass2jax, mybir

    bass2jax.install_neuronx_cc_hook()
    nc_mod = nc
    partition_name = (nc_mod.partition_id_tensor.name
                      if nc_mod.partition_id_tensor else None)
    in_names, out_names, out_avals, zero_outs = [], [], [], []
    for alloc in nc_mod.m.functions[0].allocations:
        if not isinstance(alloc, mybir.MemoryLocationSet):
            continue
        name = alloc.memorylocations[0].name
        if alloc.kind == "ExternalInput":
            if name != partition_name:
                in_names.append(name)
        elif alloc.kind == "ExternalOutput":
            out_names.append(name)
            shape = tuple(alloc.tensor_shape)
            dtype = mybir.dt.np(alloc.dtype)
            out_avals.append(jax.core.ShapedArray(shape, dtype))
            zero_outs.append(np.zeros(shape, dtype))
    n_params = len(in_names)
    all_names = in_names + out_names
    if partition_name is not None:
        all_names = all_names + [partition_name]

    def _body(*args):
        operands = list(args)
        if partition_name is not None:
            operands.append(bass2jax.partition_id_tensor())
        outs = bass2jax._bass_exec_p.bind(
            *operands,
            out_avals=tuple(out_avals),
            in_names=tuple(all_names),
            out_names=tuple(out_names),
            lowering_input_output_aliases=(),
            sim_require_finite=True,
            sim_require_nnan=True,
            nc=nc_mod,
        )
        return tuple(outs)

    devices = jax.devices()[:NCORES]
    mesh = bass2jax.Mesh(np.asarray(devices), ("core",))
    pspec = bass2jax.PartitionSpec("core")
    in_specs = (pspec,) * (n_params + len(out_names))
    out_specs = (pspec,) * len(out_names)
    sharded = jax.jit(
        bass2jax.shard_map(_body, mesh=mesh, in_specs=in_specs,
                           out_specs=out_specs, check_rep=False),
        keep_unused=True,
    )
    sharding = jax.sharding.NamedSharding(mesh, pspec)
    dev_in = [
        jax.device_put(
            np.concatenate([np.asarray(in_maps[c][n]) for c in range(NCORES)],
                           axis=0),
            sharding,
        )
        for n in in_names
    ]
    dev_zero = [
        jax.device_put(np.zeros((NCORES * z.shape[0], *z.shape[1:]), z.dtype),
                       sharding)
        for z in zero_outs
    ]
    jax.block_until_ready(dev_in + dev_zero)

    best = None
    out_arrs = None
    for _ in range(iters):
        t0 = time.perf_counter()
        out_arrs = sharded(*dev_in, *dev_zero)
        jax.block_until_ready(out_arrs)
        dt = time.perf_counter() - t0
        best = dt if best is None else min(best, dt)
    results = [
        {name: np.asarray(out_arrs[i]).reshape(NCORES, *out_avals[i].shape)[c]
         for i, name in enumerate(out_names)}
        for c in range(NCORES)
    ]
    return results, best


def kernel(**inputs):
    global LAST_EXEC_NS
    from concourse.bass_utils import run_bass_kernel_spmd

    meta, host = _plan(np.asarray(inputs["edge_index"]))
    nc = build_nc(meta)
    in_maps = make_in_maps(inputs, host)
    if os.environ.get("BASS_GCN_TIME", "0") == "1":
        results, best = run_hw_timed(nc, in_maps)
        LAST_EXEC_NS = int(best * 1e9)
        return unshard(results, host)
    res = run_bass_kernel_spmd(nc, in_maps, core_ids=list(range(NCORES)))
    LAST_EXEC_NS = res.exec_time_ns
    return unshard(res.results, host)


# revision 3
# speedup vs baseline: 1.0430x; 1.0430x over previous
"""2-layer GCN (GCNConv x2 + log_softmax) on 8 Trainium2 NeuronCores — v2.

out = log_softmax( A_hat @ relu(A_hat @ (x@W1) + b1) @ W2 + b2 )
with A_hat = D^-1/2 (A + I) D^-1/2.

v2 replaces the per-edge HBM dma_gather (GPSIMD Q7 descriptor generation was
77% of runtime) with SBUF-resident ap_gather in a FEATURE-MAJOR layout:

  * The node-feature table lives transposed in SBUF as [128, padloc] f32:
    partition 16*g+f holds feature f of core g's nodes (free dim = node).
    One AllGather of the local [16, padloc] dump materializes exactly this
    layout.
  * ap_gather runs on all 8 Q7 cores in parallel; group g (partitions
    16g..16g+15) gathers columns (= node vectors) with its own index list.
    Group g owns the edges whose SRC belongs to core g; each core owns the
    edges whose DST it holds.
  * Scatter-free segment-sum: per (core, group), dst nodes are ranked by
    in-degree; occurrence round r gathers the r-th message for ranks
    [0, n_r) which lands as a contiguous COLUMN block, so accumulation is a
    dense DVE add onto an accumulator prefix. Round sizes are maxed across
    all (core, group) instances (shortfall gathers a zeroed dead column), so
    one SPMD program serves all cores.
  * Rank->node merge: one ap_gather per group over the accumulator, then the
    8 groups are summed with a single [128]->[16] TensorE matmul (selection
    matrix B); + the self-loop row gives tot = A_hat-propagated features.
  * Both propagations run at width 16 (A_hat(zW2) == (A_hat z)W2); layer 2
    reuses identical gather indices. The final W2 matmul emits node-major
    [128, 40] tiles directly (lhsT = feature-major slice), then log_softmax.
"""

import os
import sys
from contextlib import ExitStack

import numpy as np

if "/opt/trn_rl_repo" not in sys.path:
    sys.path.insert(0, "/opt/trn_rl_repo")

# ---------------------------------------------------------------- constants
N_NODES = 100000
NCORES = 8
NGRP = 8
F_IN = 512
HID = 16
NCLS = 40
P = 128
S = 5632          # gather idxs per group per ap_gather call
SIDX = S // 16
MCOL = 448        # merge matmul column tile (psum: 448*4B = 1792B)

LAST_EXEC_NS = None


def _dims():
    nloc = N_NODES // NCORES
    tslot = -(-nloc // P)
    return nloc, tslot, tslot * P


def _wrap16(flat):
    """idx vector (len % 16 == 0) -> [16, n/16] per-group ap_gather layout."""
    n = flat.size
    assert n % 16 == 0
    return flat.reshape(n // 16, 16).T.astype(np.int16)


# ================================================================ host plan
def _plan(edge_index):
    nloc, tslot, padloc = _dims()
    dead = padloc - 1                     # zeroed pad column
    assert padloc - 1 <= np.iinfo(np.int16).max
    assert padloc > nloc

    src = np.asarray(edge_index[0]).astype(np.int64)
    dst = np.asarray(edge_index[1]).astype(np.int64)
    owner = dst // nloc
    gsrc = src // nloc
    scol = src % nloc
    ldst = dst % nloc

    entries = [[None] * NGRP for _ in range(NCORES)]
    nrounds = np.zeros((NCORES, NGRP), np.int64)
    midx = np.empty((NCORES, P, padloc // 16), np.int16)
    disT = np.zeros((NCORES, HID, padloc), np.float32)

    for c in range(NCORES):
        m = owner == c
        g_c, s_c, l_c = gsrc[m], scol[m], ldst[m]
        deg_tot = np.bincount(l_c, minlength=nloc).astype(np.float64) + 1.0
        disT[c, :, :nloc] = (1.0 / np.sqrt(deg_tot)).astype(np.float32)[None, :]

        for g in range(NGRP):
            mg = g_c == g
            s_q, l_q = s_c[mg], l_c[mg]
            deg_q = np.bincount(l_q, minlength=nloc)
            order_q = np.argsort(-deg_q, kind="stable")
            rank_of = np.empty(nloc, np.int64)
            rank_of[order_q] = np.arange(nloc)
            r_e = rank_of[l_q]
            o1 = np.argsort(r_e, kind="stable")
            rs, ss = r_e[o1], s_q[o1]
            deg_rank = deg_q[order_q]
            starts = np.zeros(nloc, np.int64)
            starts[1:] = np.cumsum(deg_rank)[:-1]
            occ = np.arange(rs.size, dtype=np.int64) - starts[rs]
            o2 = np.argsort(occ * nloc + rs, kind="stable")
            rows_sorted = ss[o2].astype(np.int16)
            n_r = (np.bincount(occ).astype(np.int64) if occ.size
                   else np.zeros(0, np.int64))
            offs = np.zeros(n_r.size + 1, np.int64)
            offs[1:] = np.cumsum(n_r)
            entries[c][g] = (rows_sorted, n_r, offs)
            nrounds[c, g] = n_r.size

            mi = np.full(padloc, dead, np.int64)
            mi[:nloc] = rank_of
            midx[c, 16 * g:16 * (g + 1)] = _wrap16(mi)

    # unified round sizes (max over cores and groups)
    R = int(nrounds.max())
    n_r_max = np.zeros(R, np.int64)
    for c in range(NCORES):
        for g in range(NGRP):
            n_r = entries[c][g][1]
            n_r_max[: n_r.size] = np.maximum(n_r_max[: n_r.size], n_r)
    t_starts = np.zeros(R + 1, np.int64)
    t_starts[1:] = np.cumsum(n_r_max)
    T = int(t_starts[R])
    n0 = int(n_r_max[0]) if R > 0 else 0

    def _chunks(total):
        """[S, S, ..., 16-aligned remainder] covering `total` positions."""
        nb = -(-total // S) if total > 0 else 0
        if nb == 0:
            return []
        return [S] * (nb - 1) + [-(-(total - (nb - 1) * S) // 16) * 16]

    # round-0 batches gather straight into ACC (col = stream position);
    # remaining rounds go through staging + prefix adds.
    r0_sizes = _chunks(n0)
    rest_sizes = _chunks(T - n0)
    R0NB = len(r0_sizes)
    batch_sizes = r0_sizes + rest_sizes
    NB = len(batch_sizes)
    starts = ([b * S for b in range(R0NB)]
              + [n0 + j * S for j in range(len(rest_sizes))])
    reals = ([min(S, n0 - b * S) for b in range(R0NB)]
             + [min(S, T - n0 - j * S) for j in range(len(rest_sizes))])
    r0_pad_end = sum(r0_sizes)
    assert r0_pad_end <= padloc

    # add schedule for rounds >= 1, per rest-batch
    rest_bounds = np.zeros(len(rest_sizes) + 1, np.int64)
    rest_bounds[1:] = n0 + np.cumsum(rest_sizes)
    rest_bounds[0] = n0
    addsched = [[] for _ in range(NB)]
    for r in range(1, R):
        t0, n = int(t_starts[r]), int(n_r_max[r])
        pos = t0
        while pos < t0 + n:
            j = int(np.searchsorted(rest_bounds, pos, side="right")) - 1
            ln = min(int(rest_bounds[j + 1]), t0 + n) - pos
            addsched[R0NB + j].append(
                (pos - t0, pos - int(rest_bounds[j]), int(ln)))
            pos += ln

    gidx = np.empty((NCORES, NB, P, SIDX), np.int16)
    for c in range(NCORES):
        for g in range(NGRP):
            rows_sorted, n_r, offs = entries[c][g]
            stream = np.full(T, dead, np.int64)
            for r in range(n_r.size):
                v = rows_sorted[offs[r]: offs[r + 1]]
                stream[t_starts[r]: t_starts[r] + v.size] = v
            for b in range(NB):
                blk = np.full(S, dead, np.int64)
                blk[: reals[b]] = stream[starts[b]: starts[b] + reals[b]]
                gidx[c, b, 16 * g:16 * (g + 1)] = _wrap16(blk)

    B = np.zeros((P, HID), np.float32)
    B[np.arange(P), np.arange(P) % HID] = 1.0

    meta = dict(NB=NB, addsched=addsched, batch_sizes=batch_sizes,
                R0NB=R0NB, r0_pad_end=r0_pad_end,
                tslot=tslot, nloc=nloc, padloc=padloc,
                # legacy keys for test.py prints
                ng=NB, batches=[(0, sched) for sched in addsched])
    host = dict(gidx=gidx, midx=midx, disT=disT, B=B)
    return meta, host


# ============================================================ device program
def _emit(tc, io, meta, reps=1):
    from concourse import mybir

    nc = tc.nc
    f32 = mybir.dt.float32
    bf16 = mybir.dt.bfloat16
    i16 = mybir.dt.int16
    nloc, tslot, padloc = meta["nloc"], meta["tslot"], meta["padloc"]
    NB, addsched = meta["NB"], meta["addsched"]
    kch = F_IN // P
    AF = mybir.ActivationFunctionType
    AL = mybir.AluOpType

    with ExitStack() as ctx:
        sb = ctx.enter_context(tc.tile_pool(name="sb", bufs=1))
        xb = ctx.enter_context(tc.tile_pool(name="xb", bufs=2))
        stg = ctx.enter_context(tc.tile_pool(name="stg", bufs=2))
        ib = ctx.enter_context(tc.tile_pool(name="ib", bufs=2))
        ps = ctx.enter_context(tc.tile_pool(name="ps", bufs=4, space="PSUM"))
        ps1 = ctx.enter_context(tc.tile_pool(name="ps1", bufs=1, space="PSUM"))
        dram = ctx.enter_context(tc.tile_pool(name="dram", bufs=1, space="DRAM"))

        # ---- persistent small tiles
        w1t = sb.tile([P, kch * HID], bf16, tag="w1")
        nc.sync.dma_start(
            w1t[:].rearrange("p (k h) -> p k h", h=HID),
            io["W1"].rearrange("(k p) h -> p k h", p=P),
        )
        w2t = sb.tile([HID, NCLS], f32, tag="w2")
        nc.sync.dma_start(w2t[:], io["W2"])
        b1T = sb.tile([HID, 1], f32, tag="b1T")
        nc.sync.dma_start(b1T[:], io["b1"].rearrange("o h -> h o"))

        FB = 4                      # final-phase node tiles per PSUM bank
        ones1 = sb.tile([1, P], f32, tag="ones1")
        nc.vector.memset(ones1[:], 1.0)
        b2s = sb.tile([1, FB * NCLS], f32, tag="b2s")
        for i in range(FB):
            nc.sync.dma_start(b2s[:, i * NCLS:(i + 1) * NCLS], io["b2"])
        b2p = ps1.tile([P, FB * NCLS], f32, tag="biasp")
        nc.tensor.matmul(b2p[:], lhsT=ones1[:], rhs=b2s[:], start=True, stop=True)
        b2bc = sb.tile([P, FB * NCLS], f32, tag="b2bc")
        nc.vector.tensor_copy(b2bc[:], b2p[:])

        Bt = sb.tile([P, HID], f32, tag="Bt")
        nc.sync.dma_start(Bt[:], io["B"])

        # Scratch bands. Verifier rules: engine AP base partition must be
        # 0/32/64/96, and two SBUF inputs of a DVE op must share the base —
        # so every two-SBUF-input op here pairs equal bases, everything else
        # goes through PSUM (mixed PSUM+SBUF inputs are exempt).
        #   rows 0:16  = hh -> z -> p  (sequential reuse, base 0)
        #   rows 64:80 = disT
        SM = sb.tile([P, padloc], f32, tag="SM")
        nc.sync.dma_start(SM[64:80, :], io["disT"])
        TB = sb.tile([P, padloc], f32, tag="TB")
        ACC = sb.tile([P, padloc], f32, tag="ACC")

        mx = sb.tile([P, tslot], f32, tag="mx")
        sm_ = sb.tile([P, tslot], f32, tag="sm")
        ls = sb.tile([P, tslot], f32, tag="ls")

        # tables travel bf16 (cast on dump, cast back on load) to halve the
        # AllGather payload; gathers and accumulation stay f32.
        hhd = dram.tile([HID, padloc], bf16, tag="hhd")
        ztd = dram.tile([HID, padloc], bf16, tag="ztd")
        table1 = dram.tile([P, padloc], bf16, tag="table1", addr_space="Shared")
        table2 = dram.tile([P, padloc], bf16, tag="table2", addr_space="Shared")

        def a3(t):
            return t[:].rearrange("p (n d) -> p n d", d=1)

        def allgather(local_dram, table_dram):
            nc.gpsimd.collective_compute(
                "AllGather", AL.bypass,
                replica_groups=[list(range(NCORES))],
                ins=[local_dram[:].opt()], outs=[table_dram[:].opt()],
            )

        for _rep in range(reps):
            # ---- phase A: hh^T = disT * (x @ W1)^T  (feature-major)
            # 512-col chunks: one [128 x cw] matmul per k-slice into a full
            # PSUM bank (4 MMs/chunk), one dis-scale DVE op per chunk.
            CW = 2 * P
            nc.vector.memset(SM[0:16, :], 0.0)
            for t0 in range(0, nloc, CW):
                cw = min(CW, nloc - t0)
                xt = xb.tile([P, kch * CW], bf16, tag="xt")
                nc.sync.dma_start(
                    xt[:, : kch * cw].rearrange("p (k n) -> p k n", k=kch),
                    io["xT"][:, t0: t0 + cw].rearrange("(k p) n -> p k n", p=P),
                )
                hp = ps.tile([HID, CW], f32, tag="u")
                for k in range(kch):
                    nc.tensor.matmul(
                        hp[:, :cw],
                        lhsT=w1t[:, k * HID: (k + 1) * HID],
                        rhs=xt[:, k * cw: (k + 1) * cw],
                        start=(k == 0),
                        stop=(k == kch - 1),
                    )
                nc.vector.tensor_tensor(
                    SM[0:16, t0: t0 + cw], hp[:, :cw],
                    SM[64:80, t0: t0 + cw], op=AL.mult)

            def propagate(layer, after_half=None):
                """Gather + segment-sum + merge; leaves the layer's output
                (z for layer 1, p for layer 2) in SM[0:16], consuming the
                self/table source previously there (hh or z). after_half
                (col_lo, col_hi) runs after each merge half's chunk chain —
                its work overlaps the other half's gather."""
                R0NB, r0_end = meta["R0NB"], meta["r0_pad_end"]
                if r0_end < padloc:
                    nc.vector.memset(ACC[:, r0_end:], 0.0)
                for b in range(NB):
                    bs = meta["batch_sizes"][b]
                    it = ib.tile([P, SIDX], i16, tag="it")
                    nc.sync.dma_start(it[:, : bs // 16],
                                      io["gidx"][b][:, : bs // 16])
                    if b < R0NB:
                        # round 0: gather straight into the accumulator
                        nc.gpsimd.ap_gather(
                            out_ap=a3(ACC[:, b * S: b * S + bs]), in_ap=a3(TB),
                            idxs_ap=it[:, : bs // 16],
                            channels=P, num_elems=padloc, d=1, num_idxs=bs,
                        )
                        continue
                    st = stg.tile([P, S], f32, tag="st")
                    nc.gpsimd.ap_gather(
                        out_ap=a3(st[:, :bs]), in_ap=a3(TB),
                        idxs_ap=it[:, : bs // 16],
                        channels=P, num_elems=padloc, d=1, num_idxs=bs,
                    )
                    for (a0, o, ln) in addsched[b]:
                        nc.vector.tensor_add(
                            ACC[:, a0:a0 + ln], ACC[:, a0:a0 + ln],
                            st[:, o:o + ln])
                # merge rank->node (ap_gather staged into TB, which is dead
                # by now), sum the 8 groups (TensorE), add self, and apply
                # the layer's dis-scaling chain per MCOL chunk. Two halves so
                # the first half's chunk chain overlaps the second gather.
                # PSUM-mixed inputs dodge the equal-base-partition rule.
                mit = ib.tile([P, padloc // 16], i16, tag="it")
                nc.sync.dma_start(mit[:], io["midx"])
                half = (padloc // 2 + 15) // 16 * 16 if padloc > 2 * MCOL else padloc
                for (m0, m1) in ([(0, half), (half, padloc)]
                                 if half < padloc else [(0, padloc)]):
                    nc.gpsimd.ap_gather(
                        out_ap=a3(TB[:, m0:m1]), in_ap=a3(ACC),
                        idxs_ap=mit[:, m0 // 16: m1 // 16],
                        channels=P, num_elems=padloc, d=1, num_idxs=m1 - m0,
                    )
                    for q in range(-(-(m1 - m0) // MCOL)):
                        c0 = m0 + q * MCOL
                        cw = min(MCOL, m1 - c0)
                        sl = slice(c0, c0 + cw)
                        pm = ps.tile([HID, MCOL], f32, tag="u")
                        nc.tensor.matmul(pm[:, :cw], lhsT=Bt[:], rhs=TB[:, sl],
                                         start=True, stop=True)
                        pm2 = ps.tile([HID, MCOL], f32, tag="u")
                        nc.vector.tensor_add(pm2[:, :cw], pm[:, :cw],
                                             SM[0:16, sl])
                        # overwrite self with dis * (merged + self)
                        nc.vector.tensor_tensor(SM[0:16, sl], pm2[:, :cw],
                                                SM[64:80, sl], op=AL.mult)
                        if layer == 1:
                            # z = dis * relu(<prev> + b1)
                            nc.scalar.activation(SM[0:16, sl], SM[0:16, sl],
                                                 AF.Relu, bias=b1T[:])
                            cpd = ps.tile([HID, MCOL], f32, tag="u")
                            nc.vector.tensor_copy(cpd[:, :cw], SM[64:80, sl])
                            nc.vector.tensor_tensor(SM[0:16, sl],
                                                    SM[0:16, sl],
                                                    cpd[:, :cw], op=AL.mult)
                    if after_half is not None:
                        after_half(m0, m1)

            # ---- layer 1  (SM[0:16]: hh -> z)
            nc.gpsimd.dma_start(hhd[:], SM[0:16, :])       # cast f32 -> bf16
            allgather(hhd, table1)
            nc.gpsimd.dma_start(TB[:], table1[:])          # cast bf16 -> f32
            propagate(1)

            # ---- layer 2  (SM[0:16]: z -> p), logits interleaved with the
            # merge halves (lg/ex live in the now-idle staging pool so the
            # final matmuls don't conflict with the merge's ACC reads)
            nc.gpsimd.dma_start(ztd[:], SM[0:16, :])
            allgather(ztd, table2)
            nc.gpsimd.dma_start(TB[:], table2[:])

            lgt = stg.tile([P, S], f32, tag="st")
            ext = stg.tile([P, S], f32, tag="st")
            lg = lgt[:, : tslot * NCLS]
            ex = ext[:, : tslot * NCLS]
            nc.vector.memset(lg, 0.0)   # rows past nloc in the last tile
                                        # are never written by emit_logits

            next_t = [0]

            def emit_logits(m0, m1):
                # tiles whose pT columns are fully merged ([0, m1))
                t_lo = next_t[0]
                t_hi = tslot if m1 >= padloc else min(tslot, m1 // P)
                next_t[0] = t_hi
                for t0 in range(t_lo, t_hi, FB):
                    tn = min(FB, t_hi - t0)
                    lp = ps.tile([P, FB * NCLS], f32, tag="u")
                    widths = [min(P, nloc - (t0 + ti) * P) for ti in range(tn)]
                    for ti in range(tn):
                        nc.tensor.matmul(
                            lp[: widths[ti], ti * NCLS:(ti + 1) * NCLS],
                            lhsT=SM[0:16,
                                    (t0 + ti) * P: (t0 + ti) * P + widths[ti]],
                            rhs=w2t[:], start=True, stop=True)
                    if all(w == P for w in widths):
                        nc.vector.tensor_add(
                            lg[:, t0 * NCLS:(t0 + tn) * NCLS],
                            lp[:, : tn * NCLS], b2bc[:, : tn * NCLS])
                    else:
                        for ti in range(tn):
                            t = t0 + ti
                            nc.vector.tensor_add(
                                lg[: widths[ti], t * NCLS:(t + 1) * NCLS],
                                lp[: widths[ti], ti * NCLS:(ti + 1) * NCLS],
                                b2bc[: widths[ti], :NCLS])

            propagate(2, after_half=emit_logits)

            lg3 = lg.rearrange("p (t c) -> p t c", c=NCLS)
            nc.vector.reduce_max(out=mx[:], in_=lg3, axis=mybir.AxisListType.X)
            nc.vector.tensor_tensor(
                lg3, lg3, mx[:].unsqueeze(2).to_broadcast([P, tslot, NCLS]),
                op=AL.subtract)
            nc.scalar.activation(ex, lg, AF.Exp)
            nc.vector.reduce_sum(
                out=sm_[:], in_=ex.rearrange("p (t c) -> p t c", c=NCLS),
                axis=mybir.AxisListType.X)
            nc.scalar.activation(ls[:], sm_[:], AF.Ln)
            nc.vector.tensor_tensor(
                lg3, lg3, ls[:].unsqueeze(2).to_broadcast([P, tslot, NCLS]),
                op=AL.subtract)
            nc.sync.dma_start(io["out_raw"], lg)


def build_nc(meta, reps=1):
    import concourse.bacc as bacc
    import concourse.tile as tile
    from concourse import mybir

    nloc, tslot, padloc = _dims()
    f32, i16 = mybir.dt.float32, mybir.dt.int16
    bf16 = mybir.dt.bfloat16
    NB = meta["NB"]

    nc = bacc.Bacc("TRN2", target_bir_lowering=False, debug=False,
                   num_devices=NCORES)
    io = {
        "xT": nc.dram_tensor("xT", [F_IN, nloc], bf16, kind="ExternalInput").ap(),
        "W1": nc.dram_tensor("W1", [F_IN, HID], bf16, kind="ExternalInput").ap(),
        "b1": nc.dram_tensor("b1", [1, HID], f32, kind="ExternalInput").ap(),
        "W2": nc.dram_tensor("W2", [HID, NCLS], f32, kind="ExternalInput").ap(),
        "b2": nc.dram_tensor("b2", [1, NCLS], f32, kind="ExternalInput").ap(),
        "disT": nc.dram_tensor("disT", [HID, padloc], f32,
                               kind="ExternalInput").ap(),
        "gidx": nc.dram_tensor("gidx", [NB, P, SIDX], i16,
                               kind="ExternalInput").ap(),
        "midx": nc.dram_tensor("midx", [P, padloc // 16], i16,
                               kind="ExternalInput").ap(),
        "B": nc.dram_tensor("B", [P, HID], f32, kind="ExternalInput").ap(),
        "out_raw": nc.dram_tensor("out_raw", [P, tslot * NCLS], f32,
                                  kind="ExternalOutput").ap(),
    }
    with tile.TileContext(nc) as tc:
        _emit(tc, io, meta, reps=reps)
    nc.compile()
    return nc


def make_in_maps(inputs, host):
    import ml_dtypes

    nloc, _, _ = _dims()
    x = np.asarray(inputs["x"], np.float32)
    W1 = np.ascontiguousarray(
        np.asarray(inputs["W1"], np.float32).astype(ml_dtypes.bfloat16))
    b1 = np.asarray(inputs["b1"], np.float32).reshape(1, HID)
    W2 = np.ascontiguousarray(np.asarray(inputs["W2"], np.float32))
    b2 = np.asarray(inputs["b2"], np.float32).reshape(1, NCLS)
    in_maps = []
    for c in range(NCORES):
        in_maps.append({
            "xT": np.ascontiguousarray(
                x[c * nloc: (c + 1) * nloc].T.astype(ml_dtypes.bfloat16)),
            "W1": W1, "b1": b1, "W2": W2, "b2": b2,
            "disT": host["disT"][c],
            "gidx": host["gidx"][c],
            "midx": host["midx"][c],
            "B": host["B"],
        })
    return in_maps


def unshard(results, host):
    nloc, tslot, padloc = _dims()
    out = np.empty((N_NODES, NCLS), np.float32)
    for c in range(NCORES):
        raw = results[c]["out_raw"].reshape(P, tslot, NCLS)
        out[c * nloc: (c + 1) * nloc] = (
            raw.transpose(1, 0, 2).reshape(padloc, NCLS)[:nloc])
    return out


def run_hw_timed(nc, in_maps, iters=6):
    """Device-resident wall-clock timing over repeated dispatches."""
    import time

    import jax
    from concourse import bass2jax, mybir

    bass2jax.install_neuronx_cc_hook()
    nc_mod = nc
    partition_name = (nc_mod.partition_id_tensor.name
                      if nc_mod.partition_id_tensor else None)
    in_names, out_names, out_avals, zero_outs = [], [], [], []
    for alloc in nc_mod.m.functions[0].allocations:
        if not isinstance(alloc, mybir.MemoryLocationSet):
            continue
        name = alloc.memorylocations[0].name
        if alloc.kind == "ExternalInput":
            if name != partition_name:
                in_names.append(name)
        elif alloc.kind == "ExternalOutput":
            out_names.append(name)
            shape = tuple(alloc.tensor_shape)
            dtype = mybir.dt.np(alloc.dtype)
            out_avals.append(jax.core.ShapedArray(shape, dtype))
            zero_outs.append(np.zeros(shape, dtype))
    n_params = len(in_names)
    all_names = in_names + out_names
    if partition_name is not None:
        all_names = all_names + [partition_name]

    def _body(*args):
        operands = list(args)
        if partition_name is not None:
            operands.append(bass2jax.partition_id_tensor())
        outs = bass2jax._bass_exec_p.bind(
            *operands,
            out_avals=tuple(out_avals),
            in_names=tuple(all_names),
            out_names=tuple(out_names),
            lowering_input_output_aliases=(),
            sim_require_finite=True,
            sim_require_nnan=True,
            nc=nc_mod,
        )
        return tuple(outs)

    devices = jax.devices()[:NCORES]
    mesh = bass2jax.Mesh(np.asarray(devices), ("core",))
    pspec = bass2jax.PartitionSpec("core")
    in_specs = (pspec,) * (n_params + len(out_names))
    out_specs = (pspec,) * len(out_names)
    sharded = jax.jit(
        bass2jax.shard_map(_body, mesh=mesh, in_specs=in_specs,
                           out_specs=out_specs, check_rep=False),
        keep_unused=True,
    )
    sharding = jax.sharding.NamedSharding(mesh, pspec)
    dev_in = [
        jax.device_put(
            np.concatenate([np.asarray(in_maps[c][n]) for c in range(NCORES)],
                           axis=0),
            sharding,
        )
        for n in in_names
    ]
    dev_zero = [
        jax.device_put(np.zeros((NCORES * z.shape[0], *z.shape[1:]), z.dtype),
                       sharding)
        for z in zero_outs
    ]
    jax.block_until_ready(dev_in + dev_zero)

    best = None
    out_arrs = None
    for _ in range(iters):
        t0 = time.perf_counter()
        out_arrs = sharded(*dev_in, *dev_zero)
        jax.block_until_ready(out_arrs)
        dt = time.perf_counter() - t0
        best = dt if best is None else min(best, dt)
    results = [
        {name: np.asarray(out_arrs[i]).reshape(NCORES, *out_avals[i].shape)[c]
         for i, name in enumerate(out_names)}
        for c in range(NCORES)
    ]
    return results, best


def kernel(**inputs):
    global LAST_EXEC_NS
    from concourse.bass_utils import run_bass_kernel_spmd

    meta, host = _plan(np.asarray(inputs["edge_index"]))
    nc = build_nc(meta)
    in_maps = make_in_maps(inputs, host)
    if os.environ.get("BASS_GCN_TIME", "0") == "1":
        results, best = run_hw_timed(nc, in_maps)
        LAST_EXEC_NS = int(best * 1e9)
        return unshard(results, host)
    res = run_bass_kernel_spmd(nc, in_maps, core_ids=list(range(NCORES)))
    LAST_EXEC_NS = res.exec_time_ns
    return unshard(res.results, host)


# revision 4
# speedup vs baseline: 1.0448x; 1.0017x over previous
"""2-layer GCN (GCNConv x2 + log_softmax) on 8 Trainium2 NeuronCores — v2.

out = log_softmax( A_hat @ relu(A_hat @ (x@W1) + b1) @ W2 + b2 )
with A_hat = D^-1/2 (A + I) D^-1/2.

v2 replaces the per-edge HBM dma_gather (GPSIMD Q7 descriptor generation was
77% of runtime) with SBUF-resident ap_gather in a FEATURE-MAJOR layout:

  * The node-feature table lives transposed in SBUF as [128, padloc] f32:
    partition 16*g+f holds feature f of core g's nodes (free dim = node).
    One AllGather of the local [16, padloc] dump materializes exactly this
    layout.
  * ap_gather runs on all 8 Q7 cores in parallel; group g (partitions
    16g..16g+15) gathers columns (= node vectors) with its own index list.
    Group g owns the edges whose SRC belongs to core g; each core owns the
    edges whose DST it holds.
  * Scatter-free segment-sum: per (core, group), dst nodes are ranked by
    in-degree; occurrence round r gathers the r-th message for ranks
    [0, n_r) which lands as a contiguous COLUMN block, so accumulation is a
    dense DVE add onto an accumulator prefix. Round sizes are maxed across
    all (core, group) instances (shortfall gathers a zeroed dead column), so
    one SPMD program serves all cores.
  * Rank->node merge: one ap_gather per group over the accumulator, then the
    8 groups are summed with a single [128]->[16] TensorE matmul (selection
    matrix B); + the self-loop row gives tot = A_hat-propagated features.
  * Both propagations run at width 16 (A_hat(zW2) == (A_hat z)W2); layer 2
    reuses identical gather indices. The final W2 matmul emits node-major
    [128, 40] tiles directly (lhsT = feature-major slice), then log_softmax.
"""

import os
import sys
from contextlib import ExitStack

import numpy as np

if "/opt/trn_rl_repo" not in sys.path:
    sys.path.insert(0, "/opt/trn_rl_repo")

# ---------------------------------------------------------------- constants
N_NODES = 100000
NCORES = 8
NGRP = 8
F_IN = 512
HID = 16
NCLS = 40
P = 128
S = 5632          # gather idxs per group per ap_gather call
SIDX = S // 16
MCOL = 448        # merge matmul column tile (psum: 448*4B = 1792B)

LAST_EXEC_NS = None


def _dims():
    nloc = N_NODES // NCORES
    tslot = -(-nloc // P)
    return nloc, tslot, tslot * P


def _wrap16(flat):
    """idx vector (len % 16 == 0) -> [16, n/16] per-group ap_gather layout."""
    n = flat.size
    assert n % 16 == 0
    return flat.reshape(n // 16, 16).T.astype(np.int16)


# ================================================================ host plan
def _plan(edge_index):
    nloc, tslot, padloc = _dims()
    dead = padloc - 1                     # zeroed pad column
    assert padloc - 1 <= np.iinfo(np.int16).max
    assert padloc > nloc

    src = np.asarray(edge_index[0]).astype(np.int64)
    dst = np.asarray(edge_index[1]).astype(np.int64)
    owner = dst // nloc
    gsrc = src // nloc
    scol = src % nloc
    ldst = dst % nloc

    entries = [[None] * NGRP for _ in range(NCORES)]
    nrounds = np.zeros((NCORES, NGRP), np.int64)
    midx = np.empty((NCORES, P, padloc // 16), np.int16)
    disT = np.zeros((NCORES, HID, padloc), np.float32)

    for c in range(NCORES):
        m = owner == c
        g_c, s_c, l_c = gsrc[m], scol[m], ldst[m]
        deg_tot = np.bincount(l_c, minlength=nloc).astype(np.float64) + 1.0
        disT[c, :, :nloc] = (1.0 / np.sqrt(deg_tot)).astype(np.float32)[None, :]

        for g in range(NGRP):
            mg = g_c == g
            s_q, l_q = s_c[mg], l_c[mg]
            deg_q = np.bincount(l_q, minlength=nloc)
            order_q = np.argsort(-deg_q, kind="stable")
            rank_of = np.empty(nloc, np.int64)
            rank_of[order_q] = np.arange(nloc)
            r_e = rank_of[l_q]
            o1 = np.argsort(r_e, kind="stable")
            rs, ss = r_e[o1], s_q[o1]
            deg_rank = deg_q[order_q]
            starts = np.zeros(nloc, np.int64)
            starts[1:] = np.cumsum(deg_rank)[:-1]
            occ = np.arange(rs.size, dtype=np.int64) - starts[rs]
            o2 = np.argsort(occ * nloc + rs, kind="stable")
            rows_sorted = ss[o2].astype(np.int16)
            n_r = (np.bincount(occ).astype(np.int64) if occ.size
                   else np.zeros(0, np.int64))
            offs = np.zeros(n_r.size + 1, np.int64)
            offs[1:] = np.cumsum(n_r)
            entries[c][g] = (rows_sorted, n_r, offs)
            nrounds[c, g] = n_r.size

            mi = np.full(padloc, dead, np.int64)
            mi[:nloc] = rank_of
            midx[c, 16 * g:16 * (g + 1)] = _wrap16(mi)

    # unified round sizes (max over cores and groups)
    R = int(nrounds.max())
    n_r_max = np.zeros(R, np.int64)
    for c in range(NCORES):
        for g in range(NGRP):
            n_r = entries[c][g][1]
            n_r_max[: n_r.size] = np.maximum(n_r_max[: n_r.size], n_r)
    t_starts = np.zeros(R + 1, np.int64)
    t_starts[1:] = np.cumsum(n_r_max)
    T = int(t_starts[R])
    n0 = int(n_r_max[0]) if R > 0 else 0

    def _chunks(total):
        """[S, S, ..., 16-aligned remainder] covering `total` positions."""
        nb = -(-total // S) if total > 0 else 0
        if nb == 0:
            return []
        return [S] * (nb - 1) + [-(-(total - (nb - 1) * S) // 16) * 16]

    # round-0 batches gather straight into ACC (col = stream position);
    # remaining rounds go through staging + prefix adds.
    r0_sizes = _chunks(n0)
    rest_sizes = _chunks(T - n0)
    R0NB = len(r0_sizes)
    batch_sizes = r0_sizes + rest_sizes
    NB = len(batch_sizes)
    starts = ([b * S for b in range(R0NB)]
              + [n0 + j * S for j in range(len(rest_sizes))])
    reals = ([min(S, n0 - b * S) for b in range(R0NB)]
             + [min(S, T - n0 - j * S) for j in range(len(rest_sizes))])
    r0_pad_end = sum(r0_sizes)
    assert r0_pad_end <= padloc

    # add schedule for rounds >= 1, per rest-batch
    rest_bounds = np.zeros(len(rest_sizes) + 1, np.int64)
    rest_bounds[1:] = n0 + np.cumsum(rest_sizes)
    rest_bounds[0] = n0
    addsched = [[] for _ in range(NB)]
    for r in range(1, R):
        t0, n = int(t_starts[r]), int(n_r_max[r])
        pos = t0
        while pos < t0 + n:
            j = int(np.searchsorted(rest_bounds, pos, side="right")) - 1
            ln = min(int(rest_bounds[j + 1]), t0 + n) - pos
            addsched[R0NB + j].append(
                (pos - t0, pos - int(rest_bounds[j]), int(ln)))
            pos += ln

    gidx = np.empty((NCORES, NB, P, SIDX), np.int16)
    for c in range(NCORES):
        for g in range(NGRP):
            rows_sorted, n_r, offs = entries[c][g]
            stream = np.full(T, dead, np.int64)
            for r in range(n_r.size):
                v = rows_sorted[offs[r]: offs[r + 1]]
                stream[t_starts[r]: t_starts[r] + v.size] = v
            for b in range(NB):
                blk = np.full(S, dead, np.int64)
                blk[: reals[b]] = stream[starts[b]: starts[b] + reals[b]]
                gidx[c, b, 16 * g:16 * (g + 1)] = _wrap16(blk)

    B = np.zeros((P, HID), np.float32)
    B[np.arange(P), np.arange(P) % HID] = 1.0

    meta = dict(NB=NB, addsched=addsched, batch_sizes=batch_sizes,
                R0NB=R0NB, r0_pad_end=r0_pad_end,
                tslot=tslot, nloc=nloc, padloc=padloc,
                # legacy keys for test.py prints
                ng=NB, batches=[(0, sched) for sched in addsched])
    host = dict(gidx=gidx, midx=midx, disT=disT, B=B)
    return meta, host


# ============================================================ device program
def _emit(tc, io, meta, reps=1):
    from concourse import mybir

    nc = tc.nc
    f32 = mybir.dt.float32
    bf16 = mybir.dt.bfloat16
    i16 = mybir.dt.int16
    nloc, tslot, padloc = meta["nloc"], meta["tslot"], meta["padloc"]
    NB, addsched = meta["NB"], meta["addsched"]
    kch = F_IN // P
    AF = mybir.ActivationFunctionType
    AL = mybir.AluOpType

    with ExitStack() as ctx:
        sb = ctx.enter_context(tc.tile_pool(name="sb", bufs=1))
        xb = ctx.enter_context(tc.tile_pool(name="xb", bufs=2))
        stg = ctx.enter_context(tc.tile_pool(name="stg", bufs=2))
        ib = ctx.enter_context(tc.tile_pool(name="ib", bufs=2))
        ps = ctx.enter_context(tc.tile_pool(name="ps", bufs=4, space="PSUM"))
        ps1 = ctx.enter_context(tc.tile_pool(name="ps1", bufs=1, space="PSUM"))
        dram = ctx.enter_context(tc.tile_pool(name="dram", bufs=1, space="DRAM"))

        # ---- persistent small tiles
        w1t = sb.tile([P, kch * HID], bf16, tag="w1")
        nc.sync.dma_start(
            w1t[:].rearrange("p (k h) -> p k h", h=HID),
            io["W1"].rearrange("(k p) h -> p k h", p=P),
        )
        w2t = sb.tile([HID, NCLS], f32, tag="w2")
        nc.sync.dma_start(w2t[:], io["W2"])
        b1T = sb.tile([HID, 1], f32, tag="b1T")
        nc.sync.dma_start(b1T[:], io["b1"].rearrange("o h -> h o"))

        FB = 4                      # final-phase node tiles per PSUM bank
        ones1 = sb.tile([1, P], f32, tag="ones1")
        nc.vector.memset(ones1[:], 1.0)
        b2s = sb.tile([1, FB * NCLS], f32, tag="b2s")
        for i in range(FB):
            nc.sync.dma_start(b2s[:, i * NCLS:(i + 1) * NCLS], io["b2"])
        b2p = ps1.tile([P, FB * NCLS], f32, tag="biasp")
        nc.tensor.matmul(b2p[:], lhsT=ones1[:], rhs=b2s[:], start=True, stop=True)
        b2bc = sb.tile([P, FB * NCLS], f32, tag="b2bc")
        nc.vector.tensor_copy(b2bc[:], b2p[:])

        Bt = sb.tile([P, HID], f32, tag="Bt")
        nc.sync.dma_start(Bt[:], io["B"])

        # Scratch bands. Verifier rules: engine AP base partition must be
        # 0/32/64/96, and two SBUF inputs of a DVE op must share the base —
        # so every two-SBUF-input op here pairs equal bases, everything else
        # goes through PSUM (mixed PSUM+SBUF inputs are exempt).
        #   rows 0:16  = hh -> z -> p  (sequential reuse, base 0)
        #   rows 64:80 = disT
        SM = sb.tile([P, padloc], f32, tag="SM")
        nc.sync.dma_start(SM[64:80, :], io["disT"])
        TB = sb.tile([P, padloc], f32, tag="TB")
        ACC = sb.tile([P, padloc], f32, tag="ACC")

        mx = sb.tile([P, tslot], f32, tag="mx")
        sm_ = sb.tile([P, tslot], f32, tag="sm")
        ls = sb.tile([P, tslot], f32, tag="ls")

        # tables travel bf16 (cast on dump, cast back on load) to halve the
        # AllGather payload; gathers and accumulation stay f32.
        # split each table exchange in two halves: the first half's dump +
        # AllGather fire as soon as those columns are final, hiding the
        # collective latency under the remaining compute.
        CW = 2 * P
        H1 = max(CW, (nloc // 2) // CW * CW)      # phase-A dump boundary
        MH = ((padloc // 2 + 15) // 16 * 16       # merge-half boundary
              if padloc > 2 * MCOL else padloc)
        t1_halves = [(0, H1), (H1, padloc)] if H1 < padloc else [(0, padloc)]
        t2_halves = [(0, MH), (MH, padloc)] if MH < padloc else [(0, padloc)]
        hhd = [dram.tile([HID, m1 - m0], bf16, tag=f"hhd{i}", name=f"hhd{i}")
               for i, (m0, m1) in enumerate(t1_halves)]
        ztd = [dram.tile([HID, m1 - m0], bf16, tag=f"ztd{i}", name=f"ztd{i}")
               for i, (m0, m1) in enumerate(t2_halves)]
        table1 = [dram.tile([P, m1 - m0], bf16, tag=f"table1{i}",
                            name=f"table1{i}", addr_space="Shared")
                  for i, (m0, m1) in enumerate(t1_halves)]
        table2 = [dram.tile([P, m1 - m0], bf16, tag=f"table2{i}",
                            name=f"table2{i}", addr_space="Shared")
                  for i, (m0, m1) in enumerate(t2_halves)]

        def a3(t):
            return t[:].rearrange("p (n d) -> p n d", d=1)

        def allgather(local_dram, table_dram):
            nc.gpsimd.collective_compute(
                "AllGather", AL.bypass,
                replica_groups=[list(range(NCORES))],
                ins=[local_dram[:].opt()], outs=[table_dram[:].opt()],
            )

        for _rep in range(reps):
            # ---- phase A: hh^T = disT * (x @ W1)^T  (feature-major)
            # 256-col chunks: one [128 x cw] matmul per k-slice into a PSUM
            # bank, one dis-scale DVE op per chunk. The first half's table
            # dump + AllGather fire mid-phase to hide collective latency.
            nc.vector.memset(SM[0:16, :], 0.0)
            for t0 in range(0, nloc, CW):
                cw = min(CW, nloc - t0)
                xt = xb.tile([P, kch * CW], bf16, tag="xt")
                nc.sync.dma_start(
                    xt[:, : kch * cw].rearrange("p (k n) -> p k n", k=kch),
                    io["xT"][:, t0: t0 + cw].rearrange("(k p) n -> p k n", p=P),
                )
                hp = ps.tile([HID, CW], f32, tag="u")
                for k in range(kch):
                    nc.tensor.matmul(
                        hp[:, :cw],
                        lhsT=w1t[:, k * HID: (k + 1) * HID],
                        rhs=xt[:, k * cw: (k + 1) * cw],
                        start=(k == 0),
                        stop=(k == kch - 1),
                    )
                nc.vector.tensor_tensor(
                    SM[0:16, t0: t0 + cw], hp[:, :cw],
                    SM[64:80, t0: t0 + cw], op=AL.mult)
                if t0 + cw == H1 and len(t1_halves) > 1:
                    nc.gpsimd.dma_start(hhd[0][:], SM[0:16, :H1])
                    allgather(hhd[0], table1[0])

            def propagate(layer, after_half=None):
                """Gather + segment-sum + merge; leaves the layer's output
                (z for layer 1, p for layer 2) in SM[0:16], consuming the
                self/table source previously there (hh or z). after_half
                (col_lo, col_hi) runs after each merge half's chunk chain —
                its work overlaps the other half's gather."""
                R0NB, r0_end = meta["R0NB"], meta["r0_pad_end"]
                if r0_end < padloc:
                    nc.vector.memset(ACC[:, r0_end:], 0.0)
                for b in range(NB):
                    bs = meta["batch_sizes"][b]
                    it = ib.tile([P, SIDX], i16, tag="it")
                    nc.sync.dma_start(it[:, : bs // 16],
                                      io["gidx"][b][:, : bs // 16])
                    if b < R0NB:
                        # round 0: gather straight into the accumulator
                        nc.gpsimd.ap_gather(
                            out_ap=a3(ACC[:, b * S: b * S + bs]), in_ap=a3(TB),
                            idxs_ap=it[:, : bs // 16],
                            channels=P, num_elems=padloc, d=1, num_idxs=bs,
                        )
                        continue
                    st = stg.tile([P, S], f32, tag="st")
                    nc.gpsimd.ap_gather(
                        out_ap=a3(st[:, :bs]), in_ap=a3(TB),
                        idxs_ap=it[:, : bs // 16],
                        channels=P, num_elems=padloc, d=1, num_idxs=bs,
                    )
                    for (a0, o, ln) in addsched[b]:
                        nc.vector.tensor_add(
                            ACC[:, a0:a0 + ln], ACC[:, a0:a0 + ln],
                            st[:, o:o + ln])
                # merge rank->node (ap_gather staged into TB, which is dead
                # by now), sum the 8 groups (TensorE), add self, and apply
                # the layer's dis-scaling chain per MCOL chunk. Two halves so
                # the first half's chunk chain overlaps the second gather.
                # PSUM-mixed inputs dodge the equal-base-partition rule.
                mit = ib.tile([P, padloc // 16], i16, tag="it")
                nc.sync.dma_start(mit[:], io["midx"])
                for (m0, m1) in t2_halves:
                    nc.gpsimd.ap_gather(
                        out_ap=a3(TB[:, m0:m1]), in_ap=a3(ACC),
                        idxs_ap=mit[:, m0 // 16: m1 // 16],
                        channels=P, num_elems=padloc, d=1, num_idxs=m1 - m0,
                    )
                    for q in range(-(-(m1 - m0) // MCOL)):
                        c0 = m0 + q * MCOL
                        cw = min(MCOL, m1 - c0)
                        sl = slice(c0, c0 + cw)
                        pm = ps.tile([HID, MCOL], f32, tag="u")
                        nc.tensor.matmul(pm[:, :cw], lhsT=Bt[:], rhs=TB[:, sl],
                                         start=True, stop=True)
                        pm2 = ps.tile([HID, MCOL], f32, tag="u")
                        nc.vector.tensor_add(pm2[:, :cw], pm[:, :cw],
                                             SM[0:16, sl])
                        # overwrite self with dis * (merged + self)
                        nc.vector.tensor_tensor(SM[0:16, sl], pm2[:, :cw],
                                                SM[64:80, sl], op=AL.mult)
                        if layer == 1:
                            # z = dis * relu(<prev> + b1)
                            nc.scalar.activation(SM[0:16, sl], SM[0:16, sl],
                                                 AF.Relu, bias=b1T[:])
                            cpd = ps.tile([HID, MCOL], f32, tag="u")
                            nc.vector.tensor_copy(cpd[:, :cw], SM[64:80, sl])
                            nc.vector.tensor_tensor(SM[0:16, sl],
                                                    SM[0:16, sl],
                                                    cpd[:, :cw], op=AL.mult)
                    if after_half is not None:
                        after_half(m0, m1)

            # ---- layer 1  (SM[0:16]: hh -> z)
            # (first-half dump + AllGather already fired mid-phase-A)
            i0 = len(t1_halves) - 1
            m0, m1 = t1_halves[i0]
            nc.gpsimd.dma_start(hhd[i0][:], SM[0:16, m0:m1])  # cast -> bf16
            allgather(hhd[i0], table1[i0])
            for i, (m0, m1) in enumerate(t1_halves):
                nc.gpsimd.dma_start(TB[:, m0:m1], table1[i][:])  # cast -> f32

            def l1_after(m0, m1):
                # dump this z half and start its AllGather; the collective
                # runs under the other merge half / the L2 prologue
                i = t2_halves.index((m0, m1))
                nc.gpsimd.dma_start(ztd[i][:], SM[0:16, m0:m1])
                allgather(ztd[i], table2[i])

            propagate(1, after_half=l1_after)

            # ---- layer 2  (SM[0:16]: z -> p), logits interleaved with the
            # merge halves (lg/ex live in the now-idle staging pool so the
            # final matmuls don't conflict with the merge's ACC reads)
            for i, (m0, m1) in enumerate(t2_halves):
                nc.gpsimd.dma_start(TB[:, m0:m1], table2[i][:])

            lgt = stg.tile([P, S], f32, tag="st")
            ext = stg.tile([P, S], f32, tag="st")
            lg = lgt[:, : tslot * NCLS]
            ex = ext[:, : tslot * NCLS]
            nc.vector.memset(lg, 0.0)   # rows past nloc in the last tile
                                        # are never written by emit_logits

            next_t = [0]

            def emit_logits(m0, m1):
                # tiles whose pT columns are fully merged ([0, m1))
                t_lo = next_t[0]
                t_hi = tslot if m1 >= padloc else min(tslot, m1 // P)
                next_t[0] = t_hi
                for t0 in range(t_lo, t_hi, FB):
                    tn = min(FB, t_hi - t0)
                    lp = ps.tile([P, FB * NCLS], f32, tag="u")
                    widths = [min(P, nloc - (t0 + ti) * P) for ti in range(tn)]
                    for ti in range(tn):
                        nc.tensor.matmul(
                            lp[: widths[ti], ti * NCLS:(ti + 1) * NCLS],
                            lhsT=SM[0:16,
                                    (t0 + ti) * P: (t0 + ti) * P + widths[ti]],
                            rhs=w2t[:], start=True, stop=True)
                    if all(w == P for w in widths):
                        nc.vector.tensor_add(
                            lg[:, t0 * NCLS:(t0 + tn) * NCLS],
                            lp[:, : tn * NCLS], b2bc[:, : tn * NCLS])
                    else:
                        for ti in range(tn):
                            t = t0 + ti
                            nc.vector.tensor_add(
                                lg[: widths[ti], t * NCLS:(t + 1) * NCLS],
                                lp[: widths[ti], ti * NCLS:(ti + 1) * NCLS],
                                b2bc[: widths[ti], :NCLS])

            propagate(2, after_half=emit_logits)

            lg3 = lg.rearrange("p (t c) -> p t c", c=NCLS)
            nc.vector.reduce_max(out=mx[:], in_=lg3, axis=mybir.AxisListType.X)
            nc.vector.tensor_tensor(
                lg3, lg3, mx[:].unsqueeze(2).to_broadcast([P, tslot, NCLS]),
                op=AL.subtract)
            nc.scalar.activation(ex, lg, AF.Exp)
            nc.vector.reduce_sum(
                out=sm_[:], in_=ex.rearrange("p (t c) -> p t c", c=NCLS),
                axis=mybir.AxisListType.X)
            nc.scalar.activation(ls[:], sm_[:], AF.Ln)
            nc.vector.tensor_tensor(
                lg3, lg3, ls[:].unsqueeze(2).to_broadcast([P, tslot, NCLS]),
                op=AL.subtract)
            nc.sync.dma_start(io["out_raw"], lg)


def build_nc(meta, reps=1):
    import concourse.bacc as bacc
    import concourse.tile as tile
    from concourse import mybir

    nloc, tslot, padloc = _dims()
    f32, i16 = mybir.dt.float32, mybir.dt.int16
    bf16 = mybir.dt.bfloat16
    NB = meta["NB"]

    nc = bacc.Bacc("TRN2", target_bir_lowering=False, debug=False,
                   num_devices=NCORES)
    io = {
        "xT": nc.dram_tensor("xT", [F_IN, nloc], bf16, kind="ExternalInput").ap(),
        "W1": nc.dram_tensor("W1", [F_IN, HID], bf16, kind="ExternalInput").ap(),
        "b1": nc.dram_tensor("b1", [1, HID], f32, kind="ExternalInput").ap(),
        "W2": nc.dram_tensor("W2", [HID, NCLS], f32, kind="ExternalInput").ap(),
        "b2": nc.dram_tensor("b2", [1, NCLS], f32, kind="ExternalInput").ap(),
        "disT": nc.dram_tensor("disT", [HID, padloc], f32,
                               kind="ExternalInput").ap(),
        "gidx": nc.dram_tensor("gidx", [NB, P, SIDX], i16,
                               kind="ExternalInput").ap(),
        "midx": nc.dram_tensor("midx", [P, padloc // 16], i16,
                               kind="ExternalInput").ap(),
        "B": nc.dram_tensor("B", [P, HID], f32, kind="ExternalInput").ap(),
        "out_raw": nc.dram_tensor("out_raw", [P, tslot * NCLS], f32,
                                  kind="ExternalOutput").ap(),
    }
    with tile.TileContext(nc) as tc:
        _emit(tc, io, meta, reps=reps)
    nc.compile()
    return nc


def make_in_maps(inputs, host):
    import ml_dtypes

    nloc, _, _ = _dims()
    x = np.asarray(inputs["x"], np.float32)
    W1 = np.ascontiguousarray(
        np.asarray(inputs["W1"], np.float32).astype(ml_dtypes.bfloat16))
    b1 = np.asarray(inputs["b1"], np.float32).reshape(1, HID)
    W2 = np.ascontiguousarray(np.asarray(inputs["W2"], np.float32))
    b2 = np.asarray(inputs["b2"], np.float32).reshape(1, NCLS)
    in_maps = []
    for c in range(NCORES):
        in_maps.append({
            "xT": np.ascontiguousarray(
                x[c * nloc: (c + 1) * nloc].T.astype(ml_dtypes.bfloat16)),
            "W1": W1, "b1": b1, "W2": W2, "b2": b2,
            "disT": host["disT"][c],
            "gidx": host["gidx"][c],
            "midx": host["midx"][c],
            "B": host["B"],
        })
    return in_maps


def unshard(results, host):
    nloc, tslot, padloc = _dims()
    out = np.empty((N_NODES, NCLS), np.float32)
    for c in range(NCORES):
        raw = results[c]["out_raw"].reshape(P, tslot, NCLS)
        out[c * nloc: (c + 1) * nloc] = (
            raw.transpose(1, 0, 2).reshape(padloc, NCLS)[:nloc])
    return out


def run_hw_timed(nc, in_maps, iters=6):
    """Device-resident wall-clock timing over repeated dispatches."""
    import time

    import jax
    from concourse import bass2jax, mybir

    bass2jax.install_neuronx_cc_hook()
    nc_mod = nc
    partition_name = (nc_mod.partition_id_tensor.name
                      if nc_mod.partition_id_tensor else None)
    in_names, out_names, out_avals, zero_outs = [], [], [], []
    for alloc in nc_mod.m.functions[0].allocations:
        if not isinstance(alloc, mybir.MemoryLocationSet):
            continue
        name = alloc.memorylocations[0].name
        if alloc.kind == "ExternalInput":
            if name != partition_name:
                in_names.append(name)
        elif alloc.kind == "ExternalOutput":
            out_names.append(name)
            shape = tuple(alloc.tensor_shape)
            dtype = mybir.dt.np(alloc.dtype)
            out_avals.append(jax.core.ShapedArray(shape, dtype))
            zero_outs.append(np.zeros(shape, dtype))
    n_params = len(in_names)
    all_names = in_names + out_names
    if partition_name is not None:
        all_names = all_names + [partition_name]

    def _body(*args):
        operands = list(args)
        if partition_name is not None:
            operands.append(bass2jax.partition_id_tensor())
        outs = bass2jax._bass_exec_p.bind(
            *operands,
            out_avals=tuple(out_avals),
            in_names=tuple(all_names),
            out_names=tuple(out_names),
            lowering_input_output_aliases=(),
            sim_require_finite=True,
            sim_require_nnan=True,
            nc=nc_mod,
        )
        return tuple(outs)

    devices = jax.devices()[:NCORES]
    mesh = bass2jax.Mesh(np.asarray(devices), ("core",))
    pspec = bass2jax.PartitionSpec("core")
    in_specs = (pspec,) * (n_params + len(out_names))
    out_specs = (pspec,) * len(out_names)
    sharded = jax.jit(
        bass2jax.shard_map(_body, mesh=mesh, in_specs=in_specs,
                           out_specs=out_specs, check_rep=False),
        keep_unused=True,
    )
    sharding = jax.sharding.NamedSharding(mesh, pspec)
    dev_in = [
        jax.device_put(
            np.concatenate([np.asarray(in_maps[c][n]) for c in range(NCORES)],
                           axis=0),
            sharding,
        )
        for n in in_names
    ]
    dev_zero = [
        jax.device_put(np.zeros((NCORES * z.shape[0], *z.shape[1:]), z.dtype),
                       sharding)
        for z in zero_outs
    ]
    jax.block_until_ready(dev_in + dev_zero)

    best = None
    out_arrs = None
    for _ in range(iters):
        t0 = time.perf_counter()
        out_arrs = sharded(*dev_in, *dev_zero)
        jax.block_until_ready(out_arrs)
        dt = time.perf_counter() - t0
        best = dt if best is None else min(best, dt)
    results = [
        {name: np.asarray(out_arrs[i]).reshape(NCORES, *out_avals[i].shape)[c]
         for i, name in enumerate(out_names)}
        for c in range(NCORES)
    ]
    return results, best


def kernel(**inputs):
    global LAST_EXEC_NS
    from concourse.bass_utils import run_bass_kernel_spmd

    meta, host = _plan(np.asarray(inputs["edge_index"]))
    nc = build_nc(meta)
    in_maps = make_in_maps(inputs, host)
    if os.environ.get("BASS_GCN_TIME", "0") == "1":
        results, best = run_hw_timed(nc, in_maps)
        LAST_EXEC_NS = int(best * 1e9)
        return unshard(results, host)
    res = run_bass_kernel_spmd(nc, in_maps, core_ids=list(range(NCORES)))
    LAST_EXEC_NS = res.exec_time_ns
    return unshard(res.results, host)
